# revision 6
# baseline (speedup 1.0000x reference)
"""Trainium2 Bass kernel v2 for the 1x1-conv attention block + groupnorm-swish.

Reference computation (B=2, C=128, spatial 16^3 -> N=4096):
    q = wq@query + bq; k = wk@key + bk; v = wv@value + bv   (per batch, [C, N])
    S[i, j] = sum_c q[c,i] k[c,j]; P = softmax_j(S)
    h[c, i] = sum_j v[c,j] P[i,j]
    x = wo@h + bo + value
    out = silu(group_norm(x))   (G=32 groups of 4 channels)

Sharding: 8 cores = 2 batches x 4 query-token chunks of 1024 (sequence
parallel). The only cross-core data is the groupnorm statistics reduction
(64 floats per batch group), done as a 3-round XOR-butterfly over
remote_dma_broadcast — the NEFF contains NO collectives, so the CC stream's
~100us init barrier (which made the old AllReduce tail finish at ~167us)
never exists.

Butterfly safety: writes that land on a core before its NEFF execution has
started are wiped, and launch skew between cores is tens of us. So every
round-r data send is gated on evidence the partner is alive: tiny
"ping" remote sem updates (one per round-partner, rotating, every 2nd loop
iteration + one right before each round's ready-wait). A send happens only
after >=1 ping from that partner arrived post-local-start, which proves the
partner started; the partner cannot finish before our data arrives (it
waits for it), so the write lands mid-execution. Inductively deadlock-free
for arbitrary skew.

Compute-side changes vs the 127us baseline:
- k-PROJECTION ELIMINATED: S^T = k_proj^T q_proj = k_raw^T (Wk^T Wq q_raw
  + Wk^T bq), so the host folds M = (Wq^T Wk) and the kernel computes
  qpp = M^T q_raw + b_qk once; QK tiles consume RAW bf16 k. Saves 8 PE
  matmuls + 4 scalar-engine PSUM copies (the ACT engine is the loop
  bottleneck: 32 x 1.07us exp).
- warm-up spin 40 -> 12 matmuls; input DMAs issued first.
- v^T projections for chunks 1-3 interleaved into the loop's PE slack.
- db2 denominator matmuls in fp32r (1 cyc/row) instead of fp32 (4).
- activation tables warmed in an order that keeps Sqrt/Silu/Ln/Exp resident.
"""

import sys
import types

import ml_dtypes
import numpy as np

try:
    import antenv.axon_hooks  # noqa: F401
except ImportError:
    import antenv

    _mod = types.ModuleType("antenv.axon_hooks")
    _hook_box = [None]
    _mod.set_axon_ntff_profile_hook = lambda h: _hook_box.__setitem__(0, h)
    _mod.get_axon_ntff_profile_hook = lambda: _hook_box[0]
    sys.modules["antenv.axon_hooks"] = _mod
    antenv.axon_hooks = _mod
    try:
        from trn_agent_boot.trn_boot import _ntff_profile_via_ctypes

        _mod.set_axon_ntff_profile_hook(
            _ntff_profile_via_ctypes("/opt/axon/libaxon_pjrt.so")
        )
    except Exception:
        pass

import concourse.tile as tile
from concourse import bacc, bass, mybir
from concourse.bass_utils import run_bass_kernel_spmd

B = 2
C = 128
N = 4096
NCORES = 8
CHUNKS = 4
NC = N // CHUNKS  # 1024 tokens per core
JT = N // 128  # 32 key tiles
G = 32
EPS = 1e-5

R = mybir.dt.float32r
F32 = mybir.dt.float32
BF16 = mybir.dt.bfloat16
AF = mybir.ActivationFunctionType
ALU = mybir.AluOpType

adddep = bass._add_dep_helper

_NC_CACHE = None


def _build():
    nc = bacc.Bacc("TRN2", target_bir_lowering=False, debug=False, num_devices=NCORES)

    q_in = nc.dram_tensor("q_in", [C, NC], BF16, kind="ExternalInput")
    k_in = nc.dram_tensor("k_in", [C, N], BF16, kind="ExternalInput")
    v_in = nc.dram_tensor("v_in", [C, N], BF16, kind="ExternalInput")
    # packed weights: [ (Wq^T Wk) | (wo@wv)^T ] bf16; per-channel vectors
    # [b_qk | bo_eff | gamma | beta | m0 | m1] fp32 (m0/m1 batch-group masks)
    w_in = nc.dram_tensor("w_in", [C, 2 * C], BF16, kind="ExternalInput")
    vecs_in = nc.dram_tensor("vecs", [C, 6], F32, kind="ExternalInput")
    y_out = nc.dram_tensor("y_out", [C, NC], F32, kind="ExternalOutput")

    ready = [nc.alloc_semaphore(f"ready{r}") for r in range(3)]
    dsem = [nc.alloc_semaphore(f"dsem{r}") for r in range(3)]
    loc_sem = nc.alloc_semaphore("bfly_loc")

    pending_waits = []  # (consumer BassInstruction | None, engine, sem, value)
    chain = [None]  # pool-engine SWDGE program-order chain
    n_preps = [0]

    def chain_to(inst):
        if chain[0] is not None:
            adddep(inst.ins, chain[0].ins, sync=True, reason="swdge order")
        chain[0] = inst

    with tile.TileContext(nc) as tc:
        with (
            tc.tile_pool(name="const", bufs=1) as const,
            tc.tile_pool(name="big", bufs=1) as big,
            tc.tile_pool(name="expp", bufs=3) as expp,
            tc.tile_pool(name="psum", bufs=2, space="PSUM") as psum,
            tc.tile_pool(name="dram", bufs=2, space="DRAM") as dram,
        ):
            # ---- input DMAs first so HBM streams during setup ----
            w_sb = const.tile([C, 2 * C], BF16)
            vecs = const.tile([C, 6], F32)
            q_raw = big.tile([C, NC], BF16)
            k_raw = big.tile([C, N], BF16)
            v_raw = big.tile([C, N], BF16)
            nc.scalar.dma_start(w_sb[:], w_in[:])
            nc.scalar.dma_start(q_raw[:], q_in[:])
            nc.scalar.dma_start(vecs[:], vecs_in[:])
            # k chunk 0 split across two queues for the earliest loop start
            nc.sync.dma_start(k_raw[:, 0:512], k_in[:, 0:512])
            nc.scalar.dma_start(k_raw[:, 512:1024], k_in[:, 512:1024])
            for qtr in range(1, 4):
                qs = slice(qtr * (N // 4), (qtr + 1) * (N // 4))
                nc.sync.dma_start(k_raw[:, qs], k_in[:, qs])
            for qtr in range(4):
                qs = slice(qtr * (N // 4), (qtr + 1) * (N // 4))
                nc.gpsimd.dma_start(v_raw[:, qs], v_in[:, qs])

            mT = w_sb[:, 0:C]          # (Wq^T Wk): qpp = mT^T @ q_raw
            w2T = w_sb[:, C : 2 * C]   # (wo@wv)^T for the v path
            bqk_sb = vecs[:, 0:1]
            boe_sb = vecs[:, 1:2]
            gamma_sb = vecs[:, 2:3]
            beta_sb = vecs[:, 3:4]
            m0_sb = vecs[:, 4:5]
            m1_sb = vecs[:, 5:6]

            # ---- PE warm-up: lift the cold-clock p-state ----
            warm_in = const.tile([C, 512], BF16)
            nc.vector.memset(warm_in[:].bitcast(mybir.dt.uint16), 0)
            warm_ps = psum.tile([C, 512], F32, tag="b1", name="warm_ps")
            for _ in range(4):
                nc.tensor.matmul(
                    warm_ps[:], warm_in[:, 0:C], warm_in[:], start=True, stop=True
                )

            # on-chip constants: all-ones (denominator), group collapse E
            # [C, G] / expand E^T [G, C] one-hot matrices
            ones_sb = const.tile([C, C], R)
            e_sb = const.tile([C, G], F32)
            et_sb = const.tile([G, C], F32)
            eps_sb = const.tile([G, 1], F32)
            nc.gpsimd.memset(ones_sb[:].bitcast(F32), 1.0)
            nc.gpsimd.memset(e_sb[:], 1.0)
            nc.gpsimd.affine_select(
                out=e_sb[:], in_=e_sb[:], compare_op=ALU.is_ge, fill=0.0,
                base=0, pattern=[[-(C // G), G]], channel_multiplier=1,
            )
            nc.gpsimd.affine_select(
                out=e_sb[:], in_=e_sb[:], compare_op=ALU.is_ge, fill=0.0,
                base=C // G - 1, pattern=[[C // G, G]], channel_multiplier=-1,
            )
            nc.gpsimd.memset(et_sb[:], 1.0)
            nc.gpsimd.affine_select(
                out=et_sb[:], in_=et_sb[:], compare_op=ALU.is_ge, fill=0.0,
                base=0, pattern=[[1, C]], channel_multiplier=-(C // G),
            )
            nc.gpsimd.affine_select(
                out=et_sb[:], in_=et_sb[:], compare_op=ALU.is_ge, fill=0.0,
                base=C // G - 1, pattern=[[-1, C]], channel_multiplier=C // G,
            )
            nc.vector.memset(eps_sb[:], EPS)

            # ---- ACT table warm: Sqrt/Silu/Ln early, Exp last (loop uses it)
            warm_sb = const.tile([G, 1], F32)
            nc.scalar.activation(out=warm_sb[:], in_=eps_sb[:], func=AF.Silu)
            nc.scalar.activation(out=warm_sb[:], in_=eps_sb[:], func=AF.Ln)
            nc.scalar.activation(out=warm_sb[:], in_=eps_sb[:], func=AF.Exp)

            # ---- qpp = (Wk^T Wq) q_raw + b_qk, in bf16 ----
            qpp_sb = big.tile([C, NC], BF16)
            qp = psum.tile([C, NC], F32, tag="st")
            for h in range(NC // 512):
                sl = slice(h * 512, (h + 1) * 512)
                nc.tensor.matmul(qp[:, sl], mT, q_raw[:, sl], start=True, stop=True)
            nc.vector.tensor_scalar(
                out=qpp_sb[:], in0=qp[:],
                scalar1=bqk_sb, scalar2=None, op0=ALU.add,
            )

            # residual + folded v-bias for own chunk (v is host-rotated so
            # own tokens sit at j=0)
            r_sb = big.tile([C, NC], F32)
            nc.vector.tensor_scalar(
                out=r_sb[:], in0=v_raw[:, 0:NC],
                scalar1=boe_sb, scalar2=None, op0=ALU.add,
            )

            v_raw3 = v_raw[:].rearrange("c (t j) -> c t j", j=128)
            vt_sb = big.tile([128, JT, C], BF16)

            def vproj(h):
                # w2-projected v^T tiles for chunk h (8 key tiles)
                for half in range(2):
                    vw = psum.tile([128, 512], F32, tag="b1", name=f"vw{h}_{half}")
                    for tt in range(4):
                        t = 8 * h + 4 * half + tt
                        nc.tensor.matmul(
                            vw[:, tt * 128 : (tt + 1) * 128],
                            v_raw3[:, t, :], w2T, start=True, stop=True,
                        )
                    nc.vector.tensor_copy(
                        vt_sb[:, 8 * h + 4 * half : 8 * h + 4 * half + 4, :], vw[:]
                    )

            def ping(r):
                delta = (1, 2)[r]
                rd = [None] * 8
                rd[delta] = (0, delta)
                png = nc.gpsimd.remote_sem_update_broadcast(
                    ready[r], loc_sem, rdests=rd
                )
                chain_to(png)
                trg = nc.gpsimd.trigger_dma(count=None)
                chain_to(trg)
                n_preps[0] += 1
                return png

            # ---- main attention loop over 32 key tiles ----
            k_raw3 = k_raw[:].rearrange("c (t j) -> c t j", j=128)
            h_ps = psum.tile([C, NC], F32, tag="h", bufs=1)
            acc_sb = big.tile([128, NC], R)

            def qk(t, st):
                for h in range(NC // 512):
                    sl = slice(h * 512, (h + 1) * 512)
                    nc.tensor.matmul(
                        st[:, sl], k_raw3[:, t, :], qpp_sb[:, sl],
                        start=True, stop=True,
                    )

            st_tiles = {}
            st_tiles[0] = psum.tile([128, NC], F32, tag="st", name="st0")
            qk(0, st_tiles[0])
            vproj(0)
            for t in range(JT):
                # v^T projections for chunk c land just before tile 8c
                tt2 = t + 5
                if tt2 % 8 == 2 and tt2 // 8 in (1, 2, 3):
                    vproj(tt2 // 8)
                if t + 1 < JT:
                    st_tiles[t + 1] = psum.tile(
                        [128, NC], F32, tag="st", name=f"st{t + 1}"
                    )
                    qk(t + 1, st_tiles[t + 1])
                exp_t = expp.tile([128, NC], R, tag="exp")
                act = nc.scalar.activation(
                    out=exp_t[:], in_=st_tiles.pop(t)[:], func=AF.Exp
                )
                if t % 2 == 0:
                    png = ping((t // 2) % 2)
                    adddep(png.ins, act.ins, sync=True, reason="pace ping")
                expb = expp.tile([128, NC], BF16, tag="expb")
                nc.gpsimd.tensor_copy(expb[:], exp_t[:])
                for h in range(NC // 512):
                    sl = slice(h * 512, (h + 1) * 512)
                    nc.tensor.matmul(
                        h_ps[:, sl], vt_sb[:, t, :], expb[:, sl],
                        start=(t == 0), stop=(t == JT - 1), skip_group_check=True,
                    )
                if t == 0:
                    nc.vector.tensor_copy(acc_sb[:], exp_t[:])
                else:
                    nc.vector.tensor_add(acc_sb[:], acc_sb[:], exp_t[:])

            # ---- 1/denominator: DVE reciprocal on cols [0:256] || scalar
            # exp(-ln(d)) on cols [256:1024] (fp32r matmul collapse) ----
            db2_ps = psum.tile([C, NC], F32, tag="st")
            for hh in range(2):
                sl = slice(hh * 512, (hh + 1) * 512)
                nc.tensor.matmul(
                    db2_ps[:, sl], ones_sb[:], acc_sb[:, sl],
                    start=True, stop=True,
                )
            dinv_sb = big.tile([C, NC], F32)
            ldb_sb = big.tile([C, 768], F32)
            nc.scalar.activation(
                out=ldb_sb[:], in_=db2_ps[:, 256:NC], func=AF.Ln
            )
            nc.scalar.activation(
                out=dinv_sb[:, 256:NC], in_=ldb_sb[:], func=AF.Exp, scale=-1.0
            )
            nc.vector.reciprocal(dinv_sb[:, 0:256], db2_ps[:, 0:256])

            # ---- x = (wo@h_unnorm) * dinv + (vres + bo_eff) ----
            x_sb = big.tile([C, NC], F32)
            nc.vector.tensor_mul(x_sb[:], h_ps[:], dinv_sb[:])
            nc.vector.tensor_add(x_sb[:], x_sb[:], r_sb[:])

            # ---- per-channel partial stats [sum, sumsq] over own tokens ----
            bstats = big.tile([C, 2, nc.vector.BN_STATS_DIM], F32)
            for hh in range(2):
                nc.vector.bn_stats(
                    out=bstats[:, hh, :], in_=x_sb[:, hh * 512 : (hh + 1) * 512]
                )
            mv = big.tile([C, nc.vector.BN_AGGR_DIM], F32)
            nc.vector.bn_aggr(out=mv[:], in_=bstats[:])
            rowstats = big.tile([C, 2], F32)
            nc.vector.tensor_copy(rowstats[:, 0:1], mv[:, 0:1])
            nc.vector.tensor_mul(rowstats[:, 1:2], mv[:, 0:1], mv[:, 0:1])
            nc.vector.tensor_add(rowstats[:, 1:2], rowstats[:, 1:2], mv[:, 1:2])

            # ---- 2-round XOR-butterfly sum of stats within the quad ----
            # On this box logical quads {0-3}/{4-7} sit on aligned physical
            # tpb quads (die-aligned), so deltas 1,2 reduce exactly the
            # 4-core batch group. (Verified by pairing probes + the rel-err
            # gate end-to-end.)
            pay = big.tile([C, 3, 2], F32)
            inbox = big.tile([C, 2, 2], F32)
            pm0 = nc.vector.tensor_copy(pay[:, 0, :], rowstats[:])
            for r, delta in enumerate((1, 2)):
                rd = [None] * 8
                rd[delta] = (0, delta)
                ping(r)  # fresh ping right before the ready wait
                snd = nc.gpsimd.remote_dma_broadcast(
                    inbox[:, r, :], pay[:, r, :], dsem[r], loc_sem, rdests=rd
                )
                chain_to(snd)
                if r == 0:
                    adddep(snd.ins, pm0.ins, sync=True, reason="payload ready")
                pending_waits.append((snd, nc.gpsimd, ready[r], 2))
                trg = nc.gpsimd.trigger_dma(count=None)
                chain_to(trg)
                n_preps[0] += 1
                add = nc.vector.tensor_add(
                    pay[:, r + 1, :], pay[:, r, :], inbox[:, r, :]
                )
                pending_waits.append((add, nc.vector, dsem[r], 2))
            tot = pay[:, 2, :]

            # ---- group mean / rstd -> per-channel scale+bias ----
            gs_ps = psum.tile([G, 2], F32, tag="b1")
            nc.tensor.matmul(gs_ps[:], e_sb[:], tot, start=True, stop=True)
            own = big.tile([G, 2], F32)
            nc.vector.tensor_copy(own[:], gs_ps[:])
            msr = big.tile([G, 2], F32)  # [mean, rstd]
            nc.vector.tensor_scalar(
                out=msr[:], in0=own[:], scalar1=1.0 / 16.0, scalar2=None,
                op0=ALU.mult,
            )
            m2 = big.tile([G, 1], F32)
            nc.vector.tensor_mul(m2[:], msr[:, 0:1], msr[:, 0:1])
            var = big.tile([G, 1], F32)
            nc.vector.tensor_sub(var[:], msr[:, 1:2], m2[:])
            lnv = big.tile([G, 1], F32)
            nc.scalar.activation(
                out=lnv[:], in_=var[:], func=AF.Ln, bias=eps_sb[:], scale=1.0
            )
            nc.scalar.activation(
                out=msr[:, 1:2], in_=lnv[:], func=AF.Exp, scale=-0.5
            )
            exp_ps = psum.tile([C, 2], F32, tag="b1")
            nc.tensor.matmul(exp_ps[:], et_sb[:], msr[:], start=True, stop=True)
            mr_sb = big.tile([C, 2], F32)
            nc.vector.tensor_copy(mr_sb[:], exp_ps[:])
            fs_sb = big.tile([C, 1], F32)
            nc.vector.tensor_mul(fs_sb[:], mr_sb[:, 1:2], gamma_sb[:])
            fb_sb = big.tile([C, 1], F32)
            nc.vector.tensor_mul(fb_sb[:], mr_sb[:, 0:1], fs_sb[:])
            nc.vector.tensor_sub(fb_sb[:], beta_sb[:], fb_sb[:])

            # ---- out = silu(fs * x + fb) ----
            y_sb = big.tile([C, NC], F32)
            for hh in range(2):
                sl = slice(hh * 512, (hh + 1) * 512)
                nc.scalar.activation(
                    out=y_sb[:, sl], in_=x_sb[:, sl], func=AF.Silu,
                    bias=fb_sb[:], scale=fs_sb[:],
                )
                nc.sync.dma_start(y_out[:, sl], y_sb[:, sl])

            # dummy collective: its presence makes the runtime launch all 8
            # cores in a coordinated way (without it, core starts skew by
            # milliseconds and the butterfly rendezvous eats the skew).
            # Never consumed; runs concurrently on the CC stream.
            cc_in = dram.tile([G, 1], F32, name="ccd_in")
            cc_out = dram.tile([G, 1], F32, name="ccd_out")
            nc.sync.dma_start(cc_in[:], eps_sb[:])
            nc.gpsimd.collective_compute(
                "AllReduce",
                ALU.add,
                replica_groups=[[0, 1, 2, 3], [4, 5, 6, 7]],
                ins=[cc_in.opt()],
                outs=[cc_out.opt()],
            )

            # drain: pool waits for all local send completions before ending
            pending_waits.append((None, nc.gpsimd, loc_sem, 16 * n_preps[0]))
            last_trigger = chain[0]

    # ---- post-tile: insert waits on remotely-incremented sems (tile's
    # scheduling sim would deadlock on them; engines execute block order) ----
    def find_pos(name):
        for fn in nc.m.functions:
            for b in fn.blocks:
                for i, ins in enumerate(b.instructions):
                    if ins.name == name:
                        return b, i
        raise KeyError(name)

    for consumer, eng, sem, val in pending_waits:
        w = eng.wait_ge(sem, val)
        wb, wi = find_pos(w.ins.name)
        wb.instructions.pop(wi)
        if consumer is not None:
            cb, ci = find_pos(consumer.ins.name)
            cb.instructions.insert(ci, w.ins)
        else:
            cb, ci = find_pos(last_trigger.ins.name)
            cb.instructions.insert(ci + 1, w.ins)

    nc.compile()
    return nc


def _get_nc():
    global _NC_CACHE
    if _NC_CACHE is None:
        _NC_CACHE = _build()
    return _NC_CACHE


def _in_maps(query, key, value, wq, bq, wk, bk, wv, bv, wo, bo, gamma, beta):
    f32 = lambda a: np.ascontiguousarray(np.asarray(a, dtype=np.float32))
    q = f32(query).reshape(B, C, N)
    k = f32(key).reshape(B, C, N)
    v = f32(value).reshape(B, C, N)
    wq, wk, wv, wo = f32(wq), f32(wk), f32(wv), f32(wo)
    bo_eff = (wo @ f32(bv).reshape(C) + f32(bo).reshape(C)).astype(np.float32)
    b_qk = (wk.T @ f32(bq).reshape(C)).astype(np.float32)

    mT = wq.T @ wk           # lhsT for qpp = (Wk^T Wq) q + b_qk
    w2 = wo @ wv             # output projection folded into the v path
    w_pack = np.concatenate([mT, w2.T], axis=1).astype(ml_dtypes.bfloat16)
    maps = []
    for p in range(NCORES):
        b, ch = divmod(p, CHUNKS)
        m0 = 1.0 if b == 0 else 0.0
        vecs = np.stack(
            [b_qk, bo_eff,
             f32(gamma).reshape(C), f32(beta).reshape(C),
             np.full(C, m0, np.float32), np.full(C, 1.0 - m0, np.float32)],
            axis=1,
        ).astype(np.float32)
        sl = slice(ch * NC, (ch + 1) * NC)
        # rotate key/value tokens so this core's chunk sits at j=0
        rot = np.roll(np.arange(N), -ch * NC)
        maps.append(
            {
                "q_in": np.ascontiguousarray(q[b][:, sl]).astype(ml_dtypes.bfloat16),
                "k_in": np.ascontiguousarray(k[b][:, rot]).astype(ml_dtypes.bfloat16),
                "v_in": np.ascontiguousarray(v[b][:, rot]).astype(ml_dtypes.bfloat16),
                "w_in": np.ascontiguousarray(w_pack),
                "vecs": np.ascontiguousarray(vecs),
            }
        )
    return maps


def kernel(query, key, value, wq, bq, wk, bk, wv, bv, wo, bo, gamma, beta):
    nc = _get_nc()
    maps = _in_maps(query, key, value, wq, bq, wk, bk, wv, bv, wo, bo, gamma, beta)
    res = run_bass_kernel_spmd(nc, maps, list(range(NCORES)))
    out = np.empty((B, C, N), dtype=np.float32)
    for p in range(NCORES):
        b, ch = divmod(p, CHUNKS)
        out[b][:, ch * NC : (ch + 1) * NC] = res.results[p]["y_out"]
    return out.reshape(B, C, 16, 16, 16)


# revision 7
# speedup vs baseline: 2.0708x; 2.0708x over previous
"""Trainium2 Bass kernel v2 for the 1x1-conv attention block + groupnorm-swish.

Reference computation (B=2, C=128, spatial 16^3 -> N=4096):
    q = wq@query + bq; k = wk@key + bk; v = wv@value + bv   (per batch, [C, N])
    S[i, j] = sum_c q[c,i] k[c,j]; P = softmax_j(S)
    h[c, i] = sum_j v[c,j] P[i,j]
    x = wo@h + bo + value
    out = silu(group_norm(x))   (G=32 groups of 4 channels)

Sharding: 8 cores = 2 batches x 4 query-token chunks of 1024 (sequence
parallel). The only cross-core data is the groupnorm statistics reduction
(64 floats per batch group), done as a 3-round XOR-butterfly over
remote_dma_broadcast — the NEFF contains NO collectives, so the CC stream's
~100us init barrier (which made the old AllReduce tail finish at ~167us)
never exists.

Butterfly safety: writes that land on a core before its NEFF execution has
started are wiped, and launch skew between cores is tens of us. So every
round-r data send is gated on evidence the partner is alive: tiny
"ping" remote sem updates (one per round-partner, rotating, every 2nd loop
iteration + one right before each round's ready-wait). A send happens only
after >=1 ping from that partner arrived post-local-start, which proves the
partner started; the partner cannot finish before our data arrives (it
waits for it), so the write lands mid-execution. Inductively deadlock-free
for arbitrary skew.

Compute-side changes vs the 127us baseline:
- k-PROJECTION ELIMINATED: S^T = k_proj^T q_proj = k_raw^T (Wk^T Wq q_raw
  + Wk^T bq), so the host folds M = (Wq^T Wk) and the kernel computes
  qpp = M^T q_raw + b_qk once; QK tiles consume RAW bf16 k. Saves 8 PE
  matmuls + 4 scalar-engine PSUM copies (the ACT engine is the loop
  bottleneck: 32 x 1.07us exp).
- warm-up spin 40 -> 12 matmuls; input DMAs issued first.
- v^T projections for chunks 1-3 interleaved into the loop's PE slack.
- db2 denominator matmuls in fp32r (1 cyc/row) instead of fp32 (4).
- activation tables warmed in an order that keeps Sqrt/Silu/Ln/Exp resident.
"""

import sys
import types

import ml_dtypes
import numpy as np

try:
    import antenv.axon_hooks  # noqa: F401
except ImportError:
    import antenv

    _mod = types.ModuleType("antenv.axon_hooks")
    _hook_box = [None]
    _mod.set_axon_ntff_profile_hook = lambda h: _hook_box.__setitem__(0, h)
    _mod.get_axon_ntff_profile_hook = lambda: _hook_box[0]
    sys.modules["antenv.axon_hooks"] = _mod
    antenv.axon_hooks = _mod
    try:
        from trn_agent_boot.trn_boot import _ntff_profile_via_ctypes

        _mod.set_axon_ntff_profile_hook(
            _ntff_profile_via_ctypes("/opt/axon/libaxon_pjrt.so")
        )
    except Exception:
        pass

import concourse.tile as tile
from concourse import bacc, bass, mybir
from concourse.bass_utils import run_bass_kernel_spmd

B = 2
C = 128
N = 4096
NCORES = 8
CHUNKS = 4
NC = N // CHUNKS  # 1024 tokens per core
JT = N // 128  # 32 key tiles
G = 32
EPS = 1e-5

R = mybir.dt.float32r
F32 = mybir.dt.float32
BF16 = mybir.dt.bfloat16
AF = mybir.ActivationFunctionType
ALU = mybir.AluOpType

adddep = bass._add_dep_helper

_NC_CACHE = None


def _build():
    nc = bacc.Bacc("TRN2", target_bir_lowering=False, debug=False, num_devices=NCORES)

    q_in = nc.dram_tensor("q_in", [C, NC], BF16, kind="ExternalInput")
    k_in = nc.dram_tensor("k_in", [C, N], BF16, kind="ExternalInput")
    v_in = nc.dram_tensor("v_in", [C, N], BF16, kind="ExternalInput")
    # packed weights: [ (Wq^T Wk) | (wo@wv)^T ] bf16; per-channel vectors
    # [b_qk | bo_eff | gamma | beta | m0 | m1] fp32 (m0/m1 batch-group masks)
    w_in = nc.dram_tensor("w_in", [C, 2 * C], BF16, kind="ExternalInput")
    vecs_in = nc.dram_tensor("vecs", [C, 6], F32, kind="ExternalInput")
    y_out = nc.dram_tensor("y_out", [C, NC], F32, kind="ExternalOutput")

    ready = [nc.alloc_semaphore(f"ready{r}") for r in range(3)]
    dsem = [nc.alloc_semaphore(f"dsem{r}") for r in range(3)]
    loc_sem = nc.alloc_semaphore("bfly_loc")

    pending_waits = []  # (consumer BassInstruction | None, engine, sem, value)
    chain = [None]  # pool-engine SWDGE program-order chain
    n_preps = [0]

    def chain_to(inst):
        if chain[0] is not None:
            adddep(inst.ins, chain[0].ins, sync=True, reason="swdge order")
        chain[0] = inst

    with tile.TileContext(nc) as tc:
        with (
            tc.tile_pool(name="const", bufs=1) as const,
            tc.tile_pool(name="big", bufs=1) as big,
            tc.tile_pool(name="expp", bufs=3) as expp,
            tc.tile_pool(name="psum", bufs=2, space="PSUM") as psum,
            tc.tile_pool(name="dram", bufs=2, space="DRAM") as dram,
        ):
            # ---- input DMAs first so HBM streams during setup ----
            w_sb = const.tile([C, 2 * C], BF16)
            vecs = const.tile([C, 6], F32)
            q_raw = big.tile([C, NC], BF16)
            k_raw = big.tile([C, N], BF16)
            v_raw = big.tile([C, N], BF16)
            nc.scalar.dma_start(w_sb[:], w_in[:])
            nc.scalar.dma_start(q_raw[:], q_in[:])
            nc.scalar.dma_start(vecs[:], vecs_in[:])
            # k chunk 0 split across two queues for the earliest loop start
            nc.sync.dma_start(k_raw[:, 0:512], k_in[:, 0:512])
            nc.scalar.dma_start(k_raw[:, 512:1024], k_in[:, 512:1024])
            for qtr in range(1, 4):
                qs = slice(qtr * (N // 4), (qtr + 1) * (N // 4))
                nc.sync.dma_start(k_raw[:, qs], k_in[:, qs])
            for qtr in range(4):
                qs = slice(qtr * (N // 4), (qtr + 1) * (N // 4))
                nc.gpsimd.dma_start(v_raw[:, qs], v_in[:, qs])

            mT = w_sb[:, 0:C]          # (Wq^T Wk): qpp = mT^T @ q_raw
            w2T = w_sb[:, C : 2 * C]   # (wo@wv)^T for the v path
            bqk_sb = vecs[:, 0:1]
            boe_sb = vecs[:, 1:2]
            gamma_sb = vecs[:, 2:3]
            beta_sb = vecs[:, 3:4]
            m0_sb = vecs[:, 4:5]
            m1_sb = vecs[:, 5:6]

            # ---- PE warm-up: lift the cold-clock p-state ----
            warm_in = const.tile([C, 512], BF16)
            nc.vector.memset(warm_in[:].bitcast(mybir.dt.uint16), 0)
            warm_ps = psum.tile([C, 512], F32, tag="b1", name="warm_ps")
            for _ in range(4):
                nc.tensor.matmul(
                    warm_ps[:], warm_in[:, 0:C], warm_in[:], start=True, stop=True
                )

            # on-chip constants: all-ones (denominator), group collapse E
            # [C, G] / expand E^T [G, C] one-hot matrices
            ones_sb = const.tile([C, C], R)
            e_sb = const.tile([C, G], F32)
            et_sb = const.tile([G, C], F32)
            eps_sb = const.tile([G, 1], F32)
            nc.gpsimd.memset(ones_sb[:].bitcast(F32), 1.0)
            nc.gpsimd.memset(e_sb[:], 1.0)
            nc.gpsimd.affine_select(
                out=e_sb[:], in_=e_sb[:], compare_op=ALU.is_ge, fill=0.0,
                base=0, pattern=[[-(C // G), G]], channel_multiplier=1,
            )
            nc.gpsimd.affine_select(
                out=e_sb[:], in_=e_sb[:], compare_op=ALU.is_ge, fill=0.0,
                base=C // G - 1, pattern=[[C // G, G]], channel_multiplier=-1,
            )
            nc.gpsimd.memset(et_sb[:], 1.0)
            nc.gpsimd.affine_select(
                out=et_sb[:], in_=et_sb[:], compare_op=ALU.is_ge, fill=0.0,
                base=0, pattern=[[1, C]], channel_multiplier=-(C // G),
            )
            nc.gpsimd.affine_select(
                out=et_sb[:], in_=et_sb[:], compare_op=ALU.is_ge, fill=0.0,
                base=C // G - 1, pattern=[[-1, C]], channel_multiplier=C // G,
            )
            nc.vector.memset(eps_sb[:], EPS)

            # ---- ACT table warm: Sqrt/Silu/Ln early, Exp last (loop uses it)
            warm_sb = const.tile([G, 1], F32)
            nc.scalar.activation(out=warm_sb[:], in_=eps_sb[:], func=AF.Silu)
            nc.scalar.activation(out=warm_sb[:], in_=eps_sb[:], func=AF.Ln)
            nc.scalar.activation(out=warm_sb[:], in_=eps_sb[:], func=AF.Exp)

            # ---- qpp = (Wk^T Wq) q_raw + b_qk, in bf16 ----
            qpp_sb = big.tile([C, NC], BF16)
            qp = psum.tile([C, NC], F32, tag="st")
            for h in range(NC // 512):
                sl = slice(h * 512, (h + 1) * 512)
                nc.tensor.matmul(qp[:, sl], mT, q_raw[:, sl], start=True, stop=True)
            nc.vector.tensor_scalar(
                out=qpp_sb[:], in0=qp[:],
                scalar1=bqk_sb, scalar2=None, op0=ALU.add,
            )

            # residual + folded v-bias for own chunk (v is host-rotated so
            # own tokens sit at j=0)
            r_sb = big.tile([C, NC], F32)
            nc.vector.tensor_scalar(
                out=r_sb[:], in0=v_raw[:, 0:NC],
                scalar1=boe_sb, scalar2=None, op0=ALU.add,
            )

            v_raw3 = v_raw[:].rearrange("c (t j) -> c t j", j=128)
            vt_sb = big.tile([128, JT, C], R)

            def vproj(h):
                # w2-projected v^T tiles for chunk h (8 key tiles)
                for half in range(2):
                    vw = psum.tile([128, 512], F32, tag="b1", name=f"vw{h}_{half}")
                    for tt in range(4):
                        t = 8 * h + 4 * half + tt
                        nc.tensor.matmul(
                            vw[:, tt * 128 : (tt + 1) * 128],
                            v_raw3[:, t, :], w2T, start=True, stop=True,
                        )
                    nc.vector.tensor_copy(
                        vt_sb[:, 8 * h + 4 * half : 8 * h + 4 * half + 4, :], vw[:]
                    )

            def ping(r):
                delta = (1, 2)[r]
                rd = [None] * 8
                rd[delta] = (0, delta)
                png = nc.gpsimd.remote_sem_update_broadcast(
                    ready[r], loc_sem, rdests=rd
                )
                chain_to(png)
                trg = nc.gpsimd.trigger_dma(count=None)
                chain_to(trg)
                n_preps[0] += 1
                return png

            # ---- main attention loop over 32 key tiles ----
            k_raw3 = k_raw[:].rearrange("c (t j) -> c t j", j=128)
            h_ps = psum.tile([C, NC], F32, tag="h", bufs=1)
            acc_sb = big.tile([128, NC], R)

            def qk(t, st):
                for h in range(NC // 512):
                    sl = slice(h * 512, (h + 1) * 512)
                    nc.tensor.matmul(
                        st[:, sl], k_raw3[:, t, :], qpp_sb[:, sl],
                        start=True, stop=True,
                    )

            st_tiles = {}
            st_tiles[0] = psum.tile([128, NC], F32, tag="st", name="st0")
            qk(0, st_tiles[0])
            vproj(0)
            for t in range(JT):
                # v^T projections for chunk c land just before tile 8c
                tt2 = t + 5
                if tt2 % 8 == 2 and tt2 // 8 in (1, 2, 3):
                    vproj(tt2 // 8)
                if t + 1 < JT:
                    st_tiles[t + 1] = psum.tile(
                        [128, NC], F32, tag="st", name=f"st{t + 1}"
                    )
                    qk(t + 1, st_tiles[t + 1])
                exp_t = expp.tile([128, NC], R, tag="exp")
                act = nc.scalar.activation(
                    out=exp_t[:], in_=st_tiles.pop(t)[:], func=AF.Exp
                )
                if t % 2 == 0:
                    png = ping((t // 2) % 2)
                    adddep(png.ins, act.ins, sync=True, reason="pace ping")
                for h in range(NC // 512):
                    sl = slice(h * 512, (h + 1) * 512)
                    nc.tensor.matmul(
                        h_ps[:, sl], vt_sb[:, t, :], exp_t[:, sl],
                        start=(t == 0), stop=(t == JT - 1), skip_group_check=True,
                    )
                if t == 0:
                    nc.vector.tensor_copy(acc_sb[:], exp_t[:])
                else:
                    nc.vector.tensor_add(acc_sb[:], acc_sb[:], exp_t[:])

            # ---- 1/denominator: DVE reciprocal on cols [0:256] || scalar
            # exp(-ln(d)) on cols [256:1024] (fp32r matmul collapse) ----
            db2_ps = psum.tile([C, NC], F32, tag="st")
            for hh in range(2):
                sl = slice(hh * 512, (hh + 1) * 512)
                nc.tensor.matmul(
                    db2_ps[:, sl], ones_sb[:], acc_sb[:, sl],
                    start=True, stop=True,
                )
            dinv_sb = big.tile([C, NC], F32)
            ldb_sb = big.tile([C, 768], F32)
            nc.scalar.activation(
                out=ldb_sb[:], in_=db2_ps[:, 256:NC], func=AF.Ln
            )
            nc.scalar.activation(
                out=dinv_sb[:, 256:NC], in_=ldb_sb[:], func=AF.Exp, scale=-1.0
            )
            nc.vector.reciprocal(dinv_sb[:, 0:256], db2_ps[:, 0:256])

            # ---- x = (wo@h_unnorm) * dinv + (vres + bo_eff) ----
            x_sb = big.tile([C, NC], F32)
            nc.vector.tensor_mul(x_sb[:], h_ps[:], dinv_sb[:])
            nc.vector.tensor_add(x_sb[:], x_sb[:], r_sb[:])

            # ---- per-channel partial stats [sum, sumsq] over own tokens ----
            bstats = big.tile([C, 2, nc.vector.BN_STATS_DIM], F32)
            for hh in range(2):
                nc.vector.bn_stats(
                    out=bstats[:, hh, :], in_=x_sb[:, hh * 512 : (hh + 1) * 512]
                )
            mv = big.tile([C, nc.vector.BN_AGGR_DIM], F32)
            nc.vector.bn_aggr(out=mv[:], in_=bstats[:])
            rowstats = big.tile([C, 2], F32)
            nc.vector.tensor_copy(rowstats[:, 0:1], mv[:, 0:1])
            nc.vector.tensor_mul(rowstats[:, 1:2], mv[:, 0:1], mv[:, 0:1])
            nc.vector.tensor_add(rowstats[:, 1:2], rowstats[:, 1:2], mv[:, 1:2])

            # ---- 2-round XOR-butterfly sum of stats within the quad ----
            # On this box logical quads {0-3}/{4-7} sit on aligned physical
            # tpb quads (die-aligned), so deltas 1,2 reduce exactly the
            # 4-core batch group. (Verified by pairing probes + the rel-err
            # gate end-to-end.)
            pay = big.tile([C, 3, 2], F32)
            inbox = big.tile([C, 2, 2], F32)
            pm0 = nc.vector.tensor_copy(pay[:, 0, :], rowstats[:])
            for r, delta in enumerate((1, 2)):
                rd = [None] * 8
                rd[delta] = (0, delta)
                ping(r)  # fresh ping right before the ready wait
                snd = nc.gpsimd.remote_dma_broadcast(
                    inbox[:, r, :], pay[:, r, :], dsem[r], loc_sem, rdests=rd
                )
                chain_to(snd)
                if r == 0:
                    adddep(snd.ins, pm0.ins, sync=True, reason="payload ready")
                pending_waits.append((snd, nc.gpsimd, ready[r], 2))
                trg = nc.gpsimd.trigger_dma(count=None)
                chain_to(trg)
                n_preps[0] += 1
                add = nc.vector.tensor_add(
                    pay[:, r + 1, :], pay[:, r, :], inbox[:, r, :]
                )
                pending_waits.append((add, nc.vector, dsem[r], 2))
            tot = pay[:, 2, :]

            # ---- group mean / rstd -> per-channel scale+bias ----
            gs_ps = psum.tile([G, 2], F32, tag="b1")
            nc.tensor.matmul(gs_ps[:], e_sb[:], tot, start=True, stop=True)
            own = big.tile([G, 2], F32)
            nc.vector.tensor_copy(own[:], gs_ps[:])
            msr = big.tile([G, 2], F32)  # [mean, rstd]
            nc.vector.tensor_scalar(
                out=msr[:], in0=own[:], scalar1=1.0 / 16.0, scalar2=None,
                op0=ALU.mult,
            )
            m2 = big.tile([G, 1], F32)
            nc.vector.tensor_mul(m2[:], msr[:, 0:1], msr[:, 0:1])
            var = big.tile([G, 1], F32)
            nc.vector.tensor_sub(var[:], msr[:, 1:2], m2[:])
            lnv = big.tile([G, 1], F32)
            nc.scalar.activation(
                out=lnv[:], in_=var[:], func=AF.Ln, bias=eps_sb[:], scale=1.0
            )
            nc.scalar.activation(
                out=msr[:, 1:2], in_=lnv[:], func=AF.Exp, scale=-0.5
            )
            exp_ps = psum.tile([C, 2], F32, tag="b1")
            nc.tensor.matmul(exp_ps[:], et_sb[:], msr[:], start=True, stop=True)
            mr_sb = big.tile([C, 2], F32)
            nc.vector.tensor_copy(mr_sb[:], exp_ps[:])
            fs_sb = big.tile([C, 1], F32)
            nc.vector.tensor_mul(fs_sb[:], mr_sb[:, 1:2], gamma_sb[:])
            fb_sb = big.tile([C, 1], F32)
            nc.vector.tensor_mul(fb_sb[:], mr_sb[:, 0:1], fs_sb[:])
            nc.vector.tensor_sub(fb_sb[:], beta_sb[:], fb_sb[:])

            # ---- out = silu(fs * x + fb) ----
            y_sb = big.tile([C, NC], F32)
            for hh in range(2):
                sl = slice(hh * 512, (hh + 1) * 512)
                nc.scalar.activation(
                    out=y_sb[:, sl], in_=x_sb[:, sl], func=AF.Silu,
                    bias=fb_sb[:], scale=fs_sb[:],
                )
                nc.sync.dma_start(y_out[:, sl], y_sb[:, sl])

            # dummy collective: its presence makes the runtime launch all 8
            # cores in a coordinated way (without it, core starts skew by
            # milliseconds and the butterfly rendezvous eats the skew).
            # Never consumed; runs concurrently on the CC stream.
            cc_in = dram.tile([G, 1], F32, name="ccd_in")
            cc_out = dram.tile([G, 1], F32, name="ccd_out")
            nc.sync.dma_start(cc_in[:], eps_sb[:])
            nc.gpsimd.collective_compute(
                "AllReduce",
                ALU.add,
                replica_groups=[[0, 1, 2, 3], [4, 5, 6, 7]],
                ins=[cc_in.opt()],
                outs=[cc_out.opt()],
            )

            # drain: pool waits for all local send completions before ending
            pending_waits.append((None, nc.gpsimd, loc_sem, 16 * n_preps[0]))
            last_trigger = chain[0]

    # ---- post-tile: insert waits on remotely-incremented sems (tile's
    # scheduling sim would deadlock on them; engines execute block order) ----
    def find_pos(name):
        for fn in nc.m.functions:
            for b in fn.blocks:
                for i, ins in enumerate(b.instructions):
                    if ins.name == name:
                        return b, i
        raise KeyError(name)

    for consumer, eng, sem, val in pending_waits:
        w = eng.wait_ge(sem, val)
        wb, wi = find_pos(w.ins.name)
        wb.instructions.pop(wi)
        if consumer is not None:
            cb, ci = find_pos(consumer.ins.name)
            cb.instructions.insert(ci, w.ins)
        else:
            cb, ci = find_pos(last_trigger.ins.name)
            cb.instructions.insert(ci + 1, w.ins)

    nc.compile()
    return nc


def _get_nc():
    global _NC_CACHE
    if _NC_CACHE is None:
        _NC_CACHE = _build()
    return _NC_CACHE


def _in_maps(query, key, value, wq, bq, wk, bk, wv, bv, wo, bo, gamma, beta):
    f32 = lambda a: np.ascontiguousarray(np.asarray(a, dtype=np.float32))
    q = f32(query).reshape(B, C, N)
    k = f32(key).reshape(B, C, N)
    v = f32(value).reshape(B, C, N)
    wq, wk, wv, wo = f32(wq), f32(wk), f32(wv), f32(wo)
    bo_eff = (wo @ f32(bv).reshape(C) + f32(bo).reshape(C)).astype(np.float32)
    b_qk = (wk.T @ f32(bq).reshape(C)).astype(np.float32)

    mT = wq.T @ wk           # lhsT for qpp = (Wk^T Wq) q + b_qk
    w2 = wo @ wv             # output projection folded into the v path
    w_pack = np.concatenate([mT, w2.T], axis=1).astype(ml_dtypes.bfloat16)
    maps = []
    for p in range(NCORES):
        b, ch = divmod(p, CHUNKS)
        m0 = 1.0 if b == 0 else 0.0
        vecs = np.stack(
            [b_qk, bo_eff,
             f32(gamma).reshape(C), f32(beta).reshape(C),
             np.full(C, m0, np.float32), np.full(C, 1.0 - m0, np.float32)],
            axis=1,
        ).astype(np.float32)
        sl = slice(ch * NC, (ch + 1) * NC)
        # rotate key/value tokens so this core's chunk sits at j=0
        rot = np.roll(np.arange(N), -ch * NC)
        maps.append(
            {
                "q_in": np.ascontiguousarray(q[b][:, sl]).astype(ml_dtypes.bfloat16),
                "k_in": np.ascontiguousarray(k[b][:, rot]).astype(ml_dtypes.bfloat16),
                "v_in": np.ascontiguousarray(v[b][:, rot]).astype(ml_dtypes.bfloat16),
                "w_in": np.ascontiguousarray(w_pack),
                "vecs": np.ascontiguousarray(vecs),
            }
        )
    return maps


def kernel(query, key, value, wq, bq, wk, bk, wv, bv, wo, bo, gamma, beta):
    nc = _get_nc()
    maps = _in_maps(query, key, value, wq, bq, wk, bk, wv, bv, wo, bo, gamma, beta)
    res = run_bass_kernel_spmd(nc, maps, list(range(NCORES)))
    out = np.empty((B, C, N), dtype=np.float32)
    for p in range(NCORES):
        b, ch = divmod(p, CHUNKS)
        out[b][:, ch * NC : (ch + 1) * NC] = res.results[p]["y_out"]
    return out.reshape(B, C, 16, 16, 16)
